# revision 1
# baseline (speedup 1.0000x reference)
"""Trainium2 Bass kernel for nn_Ensemble (dense MLP ensemble, E=8, B=65536).

Network (per ensemble member e):
    x   = concat(inputs[..., :48], clip(inputs[..., 48:64], -1, 1))   # [B, 64]
    h1  = relu(x @ W1[e] + b1[e])                                     # [B, 128]
    h2  = relu(h1 @ W2[e] + b2[e])                                    # [B, 128]
    out = h2 @ W3[e] + b3[e]                                          # [B, 48]

Sharding: ensemble dim E=8 across the 8 NeuronCores (one member per core,
weights tiny and core-resident).  Per-core layout puts features on SBUF
partitions and batch on the free dim so all three layers are weight-stationary
matmuls streaming the batch:

  - Host packs x.T into X = [128, B/2] bf16: rows 0:64   = features, batch half 0
                                             rows 64:128 = features, batch half 1
    (full 128 partitions -> full DMA port bandwidth; the action-feature clip
    is folded into the same host prep pass as the transpose/cast).
  - L1 (K=64) runs as two row-tiled matmuls (PE rows 0:64 / 64:128, concurrent).
  - L2 is a dense K=128,M=128 matmul.
  - L3 (M=48 padded to 64) runs as two col-tiled matmuls (col groups 0-1 / 2-3)
    packing two batch tiles into one PSUM bank.
  - Supersteps of 2048 batch: one big PSUM tile per layer ([128,2048] = 4
    banks), drained by a single fused bias(+relu) op: h1+out on the Scalar
    engine, h2 on the Vector engine.  Out drains [128,1024] tiles sharing the
    h2 PSUM pool slots.
"""

import numpy as np
import ml_dtypes

BF16 = ml_dtypes.bfloat16

E = 8
B = 65536
HB = B // 2          # batch half (free-dim columns per core)
IN = 64
AC = 16              # clipped action features (last 16)
H = 128
OUT = 48
OUTP = 64            # padded out features (col-group alignment)

CHUNK = 8192         # free-dim columns per x/out DMA chunk
NT = 512             # matmul free dim (one PSUM bank of fp32)
FD = 2048            # superstep psum tile width

_CACHED = None


def _build_nc(reps=None):
    """Build the bass module. reps=None -> plain kernel; reps=R wraps the
    body in a hardware For_i loop (self-timing variant)."""
    import contextlib
    import concourse.bacc as bacc
    import concourse.mybir as mybir
    import concourse.tile as tile

    f32 = mybir.dt.float32
    bf16 = mybir.dt.bfloat16
    AF = mybir.ActivationFunctionType
    ALU = mybir.AluOpType

    nc = bacc.Bacc("TRN2", target_bir_lowering=False)

    x_d = nc.dram_tensor("x", [128, HB], bf16, kind="ExternalInput")
    w1_d = nc.dram_tensor("w1p", [128, H], bf16, kind="ExternalInput")
    w2_d = nc.dram_tensor("w2", [H, H], bf16, kind="ExternalInput")
    w3_d = nc.dram_tensor("w3p", [H, OUTP], bf16, kind="ExternalInput")
    b1_d = nc.dram_tensor("b1v", [H, 1], f32, kind="ExternalInput")
    b2_d = nc.dram_tensor("b2v", [H, 1], f32, kind="ExternalInput")
    b3_d = nc.dram_tensor("b3v", [128, 1], f32, kind="ExternalInput")
    out_d = nc.dram_tensor("out", [128, HB], bf16, kind="ExternalOutput")

    with tile.TileContext(nc) as tc:
        with (
            tc.tile_pool(name="consts", bufs=1) as consts,
            tc.tile_pool(name="xp", bufs=2) as xp,
            tc.tile_pool(name="h1sb", bufs=3) as h1pool,
            tc.tile_pool(name="h2sb", bufs=3) as h2pool,
            tc.tile_pool(name="osb", bufs=2) as opool,
            tc.tile_pool(name="ps1", bufs=1, space="PSUM") as ps1,
            tc.tile_pool(name="ps2", bufs=1, space="PSUM") as ps2,
        ):
            w1_sb = consts.tile([128, H], bf16)
            w2_sb = consts.tile([H, H], bf16)
            w3_sb = consts.tile([H, OUTP], bf16)
            b1_sb = consts.tile([H, 1], f32)
            b2_sb = consts.tile([H, 1], f32)
            b3_sb = consts.tile([128, 1], f32)
            nc.sync.dma_start(out=w1_sb, in_=w1_d[:])
            nc.sync.dma_start(out=w2_sb, in_=w2_d[:])
            nc.sync.dma_start(out=w3_sb, in_=w3_d[:])
            nc.sync.dma_start(out=b1_sb, in_=b1_d[:])
            nc.sync.dma_start(out=b2_sb, in_=b2_d[:])
            nc.sync.dma_start(out=b3_sb, in_=b3_d[:])

            loop = (tc.For_i(0, reps, 1, hint_engines=(mybir.EngineType.PE,))
                    if reps is not None else contextlib.nullcontext())
            with loop:
                for c in range(HB // CHUNK):
                    x_t = xp.tile([128, CHUNK], bf16)
                    nc.sync.dma_start(out=x_t,
                                      in_=x_d[:, c * CHUNK:(c + 1) * CHUNK])
                    o_t = opool.tile([128, CHUNK], bf16)
                    for ss in range(CHUNK // (FD // 2)):
                        xc0 = ss * (FD // 2)
                        # L1: 4 row-tiled K=64 matmuls into [128, 2048]
                        h1ps = ps1.tile([128, FD], f32)
                        for i in range(FD // (2 * NT)):
                            sl = slice(xc0 + i * NT, xc0 + (i + 1) * NT)
                            nc.tensor.matmul(
                                h1ps[:, 2 * i * NT:(2 * i + 1) * NT],
                                w1_sb[0:64, :], x_t[0:64, sl],
                                start=True, stop=True)
                            nc.tensor.matmul(
                                h1ps[:, (2 * i + 1) * NT:(2 * i + 2) * NT],
                                w1_sb[64:128, :], x_t[64:128, sl],
                                start=True, stop=True)
                        h1sb = h1pool.tile([128, FD], bf16)
                        nc.scalar.activation(h1sb, h1ps, AF.Relu, bias=b1_sb)
                        # L2: dense 128x128
                        h2ps = ps2.tile([128, FD], f32, tag="h2ps")
                        for i in range(FD // NT):
                            nc.tensor.matmul(h2ps[:, i * NT:(i + 1) * NT],
                                             w2_sb, h1sb[:, i * NT:(i + 1) * NT],
                                             start=True, stop=True)
                        h2sb = h2pool.tile([128, FD], bf16)
                        nc.vector.tensor_scalar(h2sb, h2ps, b2_sb, 0.0,
                                                op0=ALU.add, op1=ALU.max)
                        # L3: 4 col-tiled M=64 matmuls, two batch tiles per
                        # PSUM bank; out tile shares the h2 psum pool slot
                        ops = ps2.tile([128, FD // 2], f32, tag="h2ps")
                        for i in range(FD // (2 * NT)):
                            nc.tensor.matmul(
                                ops[0:OUTP, i * NT:(i + 1) * NT], w3_sb,
                                h2sb[:, 2 * i * NT:(2 * i + 1) * NT],
                                start=True, stop=True, tile_position=(0, 0))
                            nc.tensor.matmul(
                                ops[OUTP:128, i * NT:(i + 1) * NT], w3_sb,
                                h2sb[:, (2 * i + 1) * NT:(2 * i + 2) * NT],
                                start=True, stop=True, tile_position=(0, OUTP))
                        nc.scalar.activation(o_t[:, xc0:xc0 + FD // 2], ops,
                                             AF.Identity, bias=b3_sb)
                    nc.sync.dma_start(out=out_d[:, c * CHUNK:(c + 1) * CHUNK],
                                      in_=o_t)

    nc.compile()
    return nc


def _get_nc():
    global _CACHED
    if _CACHED is None:
        _CACHED = _build_nc()
    return _CACHED


def _prep_member(x_e, W1_e, b1_e, W2_e, b2_e, W3_e, b3_e):
    """Host-side shard prep: transpose to feature-major, pack the two batch
    halves on the partition axis, clip action features, cast to bf16."""
    xt = np.ascontiguousarray(np.asarray(x_e).T)      # [64, B] f32
    np.clip(xt[IN - AC:IN], -1.0, 1.0, out=xt[IN - AC:IN])
    X = np.empty((128, HB), dtype=BF16)
    X[0:64] = xt[:, :HB]
    X[64:128] = xt[:, HB:]

    w1p = np.empty((128, H), dtype=BF16)
    w1p[0:64] = W1_e
    w1p[64:128] = W1_e
    w2 = W2_e.astype(BF16)
    w3p = np.zeros((H, OUTP), dtype=BF16)
    w3p[:, :OUT] = W3_e
    b1v = np.ascontiguousarray(b1_e.astype(np.float32).reshape(H, 1))
    b2v = np.ascontiguousarray(b2_e.astype(np.float32).reshape(H, 1))
    b3v = np.zeros((128, 1), dtype=np.float32)
    b3v[0:OUT, 0] = b3_e
    b3v[OUTP:OUTP + OUT, 0] = b3_e
    return {"x": X, "w1p": w1p, "w2": w2, "w3p": w3p,
            "b1v": b1v, "b2v": b2v, "b3v": b3v}


def kernel(**inputs):
    from concourse.bass_utils import run_bass_kernel_spmd

    x = np.asarray(inputs["inputs"], dtype=np.float32).reshape(E, B, IN)
    W1 = np.asarray(inputs["W1"], dtype=np.float32)
    b1 = np.asarray(inputs["b1"], dtype=np.float32)
    W2 = np.asarray(inputs["W2"], dtype=np.float32)
    b2 = np.asarray(inputs["b2"], dtype=np.float32)
    W3 = np.asarray(inputs["W3"], dtype=np.float32)
    b3 = np.asarray(inputs["b3"], dtype=np.float32)

    in_maps = [
        _prep_member(x[e], W1[e], b1[e], W2[e], b2[e], W3[e], b3[e])
        for e in range(E)
    ]

    nc = _get_nc()
    res = run_bass_kernel_spmd(nc, in_maps, core_ids=list(range(E)))

    out = np.empty((E, B, OUT), dtype=np.float32)
    for e in range(E):
        dev = res.results[e]["out"]          # [128, HB] bf16
        out[e, :HB] = dev[0:OUT, :].T
        out[e, HB:] = dev[OUTP:OUTP + OUT, :].T
    return out

